# revision 3
# baseline (speedup 1.0000x reference)
"""Trainium2 Bass kernel for nn_MultiHeadAttention_81673098101666.

Reference computation (per batch b):
    qkv  = seq @ w_qkv.T ; q,k,v = split(qkv)        # seq [S,128], q/k/v [S,1024]
    scores = q @ k.T / 32 ; attn = softmax(scores)
    out  = attn @ v @ w_out.T + b_out                # [S, 128]

Key algebraic identity (INPUT_DIM=128 => rank-128 attention):
    scoresT = A^T-contracted against seq_q   with A = M^T seqT, M = Wk^T Wq
    outT    = W2T^T (seqT E^T) / sumexp      with W2T = Wv^T Wout^T
so the S^2-sized matmuls contract over 128 dims instead of 1024 and Q/K/V
are never materialized.  A, W2T, the 1/sumexp division and the bias are all
folded on the HOST (A is a [2048,128]@[128,128] per batch - cheap), so the
device does only: scores matmuls, exp, the C = seqT E^T accumulation, and
a bf16 partial-sum of E for the softmax denominator.

Sharding: 8 cores = 4 batches x 2 query-halves; no collectives.

Performance design (per core):
  - hard floor: the exp chain on the scalar engine - 2M elements at
    1 elem/cycle/lane = ~16.2us, strictly serial.  Everything else (PE
    matmuls ~14us, DVE adds, DMA) overlaps underneath it.
  - each input DMA pays ~1us completion-semaphore latency after its last
    byte, so chain-critical transfers (at0/sqa/sqb) are small, first, and
    spread over all three rings to dodge SDMA contention.
  - first/last key-tiles' exp split into [128,512] halves: EXP0a needs only
    at0 (64KB) + sqa (128KB); EXP15a lets the final C matmuls + copies
    begin before the chain's last ACT retires.
  - warm-up matmuls (memset on gpsimd, which boots earliest) keep PE busy
    from ~6.6us so the HAM clock gate (1.2 -> 2.4 GHz) releases before the
    scores stream; C matmuls are emitted one kt behind the scores matmuls
    so the chain always has PE priority.
  - sumexp: exp tiles written into [128,2048] PAIRS; DVE accumulates pairs
    with wide adds, folds early, and only two 512-wide adds (et15 halves)
    remain after the last ACT.  The [128,1024] bf16 partial sum accF is
    DMA'd out raw; the host does the 128-row reduction.
  - tail: C0 copy+DMA on scalar (idle after the chain), C1 copy on vector
    with DMA on sync, accF on the gpsimd ring - three rings in parallel.
"""

import numpy as np

B, S, DIN = 4, 2048, 128
O = 1024
QPC = S // 2           # queries per core = 1024
QC = 512               # query-chunk width (PSUM bank limit: 512 fp32)
NKT = S // 128         # 16 key tiles
SCALE = 1.0 / 32.0     # 1/sqrt(O)

_NC = None
PROFILE = False
LAST_RESULTS = None


def _body(ctx, tc, ins, outT_d, accf_d):
    import concourse.mybir as mybir

    nc = tc.nc
    f32 = mybir.dt.float32
    b16 = mybir.dt.bfloat16
    Exp = mybir.ActivationFunctionType.Exp
    add = mybir.AluOpType.add

    consts = ctx.enter_context(tc.tile_pool(name="consts", bufs=1))
    et_pool = ctx.enter_context(tc.tile_pool(name="et", bufs=9))
    acc_pool = ctx.enter_context(tc.tile_pool(name="accp", bufs=2))
    c_pool = ctx.enter_context(tc.tile_pool(name="cp", bufs=2))
    psum = ctx.enter_context(tc.tile_pool(name="psum", bufs=1, space="PSUM"))

    # ---- SBUF tiles ----------------------------------------------------
    AT_sb = consts.tile([128, S], b16)      # A = M^T seqT  (all 2048 keys)
    SQ_sb = consts.tile([128, QPC], b16)    # this core's query half (seqT)
    SN_sb = consts.tile([128, S], b16)      # keys natural tiled, [p, t*128+i]
    warm_sb = consts.tile([128, QC], b16)

    # warm-up tile init on gpsimd - it boots earliest, so PE warm-ups can
    # start right after the preamble
    nc.gpsimd.memset(warm_sb[:], 1.0)

    # ---- input DMAs: chain-critical first, spread across rings ---------
    nc.scalar.dma_start(AT_sb[:, 0:256], ins["at0"])
    nc.sync.dma_start(SQ_sb[:, 0:QC], ins["sqa"])
    nc.scalar.dma_start(SQ_sb[:, QC:QPC], ins["sqb"])
    nc.sync.dma_start(AT_sb[:, 256:1024], ins["at1"])
    nc.gpsimd.dma_start(SN_sb[:, 0:256], ins["sna"])
    nc.scalar.dma_start(AT_sb[:, 1024:2048], ins["at2"])
    nc.sync.dma_start(SN_sb[:, 1024:2048], ins["snc"])
    nc.gpsimd.dma_start(SN_sb[:, 256:1024], ins["snb"])

    # warm-up matmuls: keep PE busy through the DMA head so the HAM
    # clock-gate releases (1.2 -> 2.4 GHz) before the real matmul stream
    for w in range(5):
        pw = psum.tile([128, QC], f32, tag="mm", bufs=3, name=f"pw{w}")
        nc.tensor.matmul(pw[:], warm_sb[:, 0:128], warm_sb[:],
                         start=True, stop=True)

    # ---- C accumulation banks ------------------------------------------
    pcs = [psum.tile([128, QC], f32, tag="ctx", bufs=2, name=f"pc{qc}")
           for qc in range(2)]

    def score_half(kt, qc, et_dst, name):
        # single [128,512] scores matmul + exp (first / last key tile)
        pp = psum.tile([128, QC], f32, tag="mm", bufs=3, name=name)
        nc.tensor.matmul(pp[:], AT_sb[:, kt * 128:(kt + 1) * 128],
                         SQ_sb[:, qc * QC:(qc + 1) * QC],
                         start=True, stop=True)
        nc.scalar.activation(et_dst, pp[:], Exp, scale=float(SCALE))

    def score_tile(kt, et_dst):
        pp = psum.tile([128, 1024], f32, tag="mm", bufs=3, name=f"pp{kt}")
        for qc in range(2):
            nc.tensor.matmul(pp[:, qc * QC:(qc + 1) * QC],
                             AT_sb[:, kt * 128:(kt + 1) * 128],
                             SQ_sb[:, qc * QC:(qc + 1) * QC],
                             start=True, stop=True, skip_group_check=True)
        nc.scalar.activation(et_dst, pp[:], Exp, scale=float(SCALE))

    def c_mm(kt, et_sl, first=False, last=False):
        for qc in range(2):
            nc.tensor.matmul(pcs[qc][:], SN_sb[:, kt * 128:(kt + 1) * 128],
                             et_sl[:, qc * QC:(qc + 1) * QC],
                             start=first, stop=last)

    # et storage: kt0 / kt15 as standalone [128,1024]; kt1..14 as pairs
    et0 = et_pool.tile([128, 1024], b16, tag="et", name="et0")
    et15 = et_pool.tile([128, 1024], b16, tag="et", name="et15")
    prs = [et_pool.tile([128, 2048], b16, tag="et", name=f"etp{p}")
           for p in range(7)]
    esl = {0: et0, 15: et15}
    for kt in range(1, 15):
        p, half = (kt - 1) // 2, (kt - 1) % 2
        esl[kt] = prs[p][:, half * 1024:(half + 1) * 1024]

    accP = acc_pool.tile([128, 2048], b16, tag="acc", name="accP")
    accF = acc_pool.tile([128, 1024], b16, tag="acc", name="accF")

    # ---- main stream: scores/exp lead, C matmuls lag one kt ------------
    score_half(0, 0, et0[:, 0:QC], "pp0a")
    score_half(0, 1, et0[:, QC:1024], "pp0b")
    for kt in range(1, 15):
        score_tile(kt, esl[kt])
        c_mm(kt - 1, esl[kt - 1], first=(kt == 1))
        # DVE pair-accumulation, woven in as pairs complete (covers et0..12)
        if kt == 5:
            nc.vector.tensor_tensor(accP[:], prs[0][:], prs[1][:], add)
            nc.vector.tensor_tensor(accP[:, 0:1024], accP[:, 0:1024],
                                    et0[:], add)
        elif kt in (7, 9, 11, 13):
            nc.vector.tensor_tensor(accP[:], accP[:], prs[(kt - 3) // 2][:],
                                    add)
    score_half(15, 0, et15[:, 0:QC], "pp15a")
    c_mm(14, esl[14])
    nc.tensor.matmul(pcs[0][:], SN_sb[:, 15 * 128:S], et15[:, 0:QC],
                     start=False, stop=True)
    score_half(15, 1, et15[:, QC:1024], "pp15b")
    nc.tensor.matmul(pcs[1][:], SN_sb[:, 15 * 128:S], et15[:, QC:1024],
                     start=False, stop=True)

    # ---- sumexp partials: fold early; only et15 halves trail the chain -
    nc.vector.tensor_tensor(accF[:], accP[:, 0:1024], accP[:, 1024:2048], add)
    nc.vector.tensor_tensor(accF[:], accF[:], prs[6][:, 0:1024], add)   # et13
    nc.vector.tensor_tensor(accF[:], accF[:], prs[6][:, 1024:2048], add)  # et14
    nc.vector.tensor_tensor(accF[:, 0:QC], accF[:, 0:QC], et15[:, 0:QC], add)
    nc.vector.tensor_tensor(accF[:, QC:1024], accF[:, QC:1024],
                            et15[:, QC:1024], add)
    nc.gpsimd.dma_start(accf_d[:], accF[:])

    # ---- outputs: C halves on scalar/vector + their two rings ----------
    C0_sb = c_pool.tile([128, QC], b16, tag="c", name="C0")
    nc.scalar.copy(C0_sb[:], pcs[0][:])
    nc.scalar.dma_start(outT_d[:, 0:QC], C0_sb[:])

    C1_sb = c_pool.tile([128, QC], b16, tag="c", name="C1")
    nc.vector.tensor_copy(C1_sb[:], pcs[1][:])
    nc.sync.dma_start(outT_d[:, QC:2 * QC], C1_sb[:])


def _build_nc():
    from contextlib import ExitStack

    import concourse.mybir as mybir
    import concourse.tile as tile
    from concourse import bacc

    b16 = mybir.dt.bfloat16
    nc = bacc.Bacc("TRN2", target_bir_lowering=False, debug=False, num_devices=8)
    shapes = {
        "at0": [128, 256], "at1": [128, 768], "at2": [128, 1024],
        "sqa": [128, QC], "sqb": [128, QC],
        "sna": [128, 256], "snb": [128, 768], "snc": [128, 1024],
    }
    ins = {k: nc.dram_tensor(k, sh, b16, kind="ExternalInput").ap()
           for k, sh in shapes.items()}
    outT_d = nc.dram_tensor("outT", [128, QPC], b16, kind="ExternalOutput").ap()
    accf_d = nc.dram_tensor("accf", [128, QPC], b16, kind="ExternalOutput").ap()

    with tile.TileContext(nc) as tc:
        with ExitStack() as ctx:
            _body(ctx, tc, ins, outT_d, accf_d)
    nc.compile()
    return nc


def get_nc():
    global _NC
    if _NC is None:
        _NC = _build_nc()
    return _NC


def make_in_maps(sequence, w_qkv):
    import ml_dtypes

    bf16 = ml_dtypes.bfloat16
    wq, wk = w_qkv[:O], w_qkv[O:2 * O]
    M = wk.T @ wq                                     # [128, 128]

    in_maps = []
    for b in range(B):
        seq = sequence[b]                             # [2048, 128] fp32
        AT = np.ascontiguousarray((seq @ M).T.astype(bf16))   # [128, 2048]
        seq16 = seq.astype(bf16)
        seqT = np.ascontiguousarray(seq16.T)          # [128, 2048]
        # seqn tiled: partition p holds [t, i] for key t*128+p
        seqn = np.ascontiguousarray(
            seq16.reshape(NKT, 128, 128).transpose(1, 0, 2).reshape(128, S))
        at0 = np.ascontiguousarray(AT[:, 0:256])
        at1 = np.ascontiguousarray(AT[:, 256:1024])
        at2 = np.ascontiguousarray(AT[:, 1024:2048])
        sna = np.ascontiguousarray(seqn[:, 0:256])
        snb = np.ascontiguousarray(seqn[:, 256:1024])
        snc = np.ascontiguousarray(seqn[:, 1024:2048])
        for h in range(2):
            in_maps.append({
                "at0": at0, "at1": at1, "at2": at2,
                "sqa": np.ascontiguousarray(seqT[:, h * QPC:h * QPC + QC]),
                "sqb": np.ascontiguousarray(seqT[:, h * QPC + QC:(h + 1) * QPC]),
                "sna": sna, "snb": snb, "snc": snc,
            })
    return in_maps


def kernel(sequence, w_qkv, w_out, b_out):
    global LAST_RESULTS
    from concourse.bass_utils import run_bass_kernel_spmd

    sequence = np.asarray(sequence, dtype=np.float32)
    w_qkv = np.asarray(w_qkv, dtype=np.float32)
    w_out = np.asarray(w_out, dtype=np.float32)
    b_out = np.asarray(b_out, dtype=np.float32)

    nc = get_nc()
    in_maps = make_in_maps(sequence, w_qkv)
    kw = {}
    if PROFILE:
        kw = dict(trace=True, trace_cores=[0])
    res = run_bass_kernel_spmd(nc, in_maps, list(range(8)), **kw)
    LAST_RESULTS = res

    wv = w_qkv[2 * O:]
    W2T = (wv.T @ w_out.T).astype(np.float32)              # [128, 128]
    out = np.empty((B, S, DIN), np.float32)
    for c in range(8):
        b, h = c // 2, c % 2
        C = res.results[c]["outT"].astype(np.float32)      # [128,1024] seqT E^T
        se = res.results[c]["accf"].astype(np.float32).sum(axis=0)  # [1024]
        outT = W2T.T @ C                                   # [128, 1024]
        out[b, h * QPC:(h + 1) * QPC, :] = outT.T / se[:, None] + b_out[None, :]
    return out


# revision 4
# speedup vs baseline: 1.1039x; 1.1039x over previous
"""Trainium2 Bass kernel for nn_MultiHeadAttention_81673098101666.

Reference computation (per batch b):
    qkv  = seq @ w_qkv.T ; q,k,v = split(qkv)        # seq [S,128], q/k/v [S,1024]
    scores = q @ k.T / 32 ; attn = softmax(scores)
    out  = attn @ v @ w_out.T + b_out                # [S, 128]

Key algebraic identity (INPUT_DIM=128 => rank-128 attention):
    scoresT = A^T-contracted against seq_q   with A = M^T seqT, M = Wk^T Wq
    outT    = W2T^T (seqT E^T) / sumexp      with W2T = Wv^T Wout^T
so the S^2-sized matmuls contract over 128 dims instead of 1024 and Q/K/V
are never materialized.  A, W2T, the 1/sumexp division and the bias are all
folded on the HOST (A is a [2048,128]@[128,128] per batch - cheap), so the
device does only: scores matmuls, exp, the C = seqT E^T accumulation, and
a bf16 partial-sum of E for the softmax denominator.

Sharding: 8 cores = 4 batches x 2 query-halves; no collectives.

Performance design (per core):
  - hard floor: the exp chain on the scalar engine - 2M elements at
    1 elem/cycle/lane = ~16.2us, strictly serial.  Everything else (PE
    matmuls ~14us, DVE adds, DMA) overlaps underneath it.
  - each input DMA pays ~1us completion-semaphore latency after its last
    byte, so chain-critical transfers (at0/sqa/sqb) are small, first, and
    spread over all three rings to dodge SDMA contention.
  - first/last key-tiles' exp split into [128,512] halves: EXP0a needs only
    at0 (64KB) + sqa (128KB); EXP15a lets the final C matmuls + copies
    begin before the chain's last ACT retires.
  - warm-up matmuls (memset on gpsimd, which boots earliest) keep PE busy
    from ~6.6us so the HAM clock gate (1.2 -> 2.4 GHz) releases before the
    scores stream; C matmuls are emitted one kt behind the scores matmuls
    so the chain always has PE priority.
  - sumexp: exp tiles written into [128,2048] PAIRS; DVE accumulates pairs
    with wide adds, folds early, and only two 512-wide adds (et15 halves)
    remain after the last ACT.  The [128,1024] bf16 partial sum accF is
    DMA'd out raw; the host does the 128-row reduction.
  - tail: C0 copy+DMA on scalar (idle after the chain), C1 copy on vector
    with DMA on sync, accF on the gpsimd ring - three rings in parallel.
"""

import numpy as np

B, S, DIN = 4, 2048, 128
O = 1024
QPC = S // 2           # queries per core = 1024
QC = 512               # query-chunk width (PSUM bank limit: 512 fp32)
NKT = S // 128         # 16 key tiles
SCALE = 1.0 / 32.0     # 1/sqrt(O)

_NC = None
PROFILE = False
LAST_RESULTS = None


def _body(ctx, tc, ins, outT_d, accf_d):
    import concourse.mybir as mybir

    nc = tc.nc
    f32 = mybir.dt.float32
    b16 = mybir.dt.bfloat16
    Exp = mybir.ActivationFunctionType.Exp
    add = mybir.AluOpType.add

    consts = ctx.enter_context(tc.tile_pool(name="consts", bufs=1))
    et_pool = ctx.enter_context(tc.tile_pool(name="et", bufs=9))
    acc_pool = ctx.enter_context(tc.tile_pool(name="accp", bufs=2))
    c_pool = ctx.enter_context(tc.tile_pool(name="cp", bufs=2))
    psum = ctx.enter_context(tc.tile_pool(name="psum", bufs=1, space="PSUM"))

    # ---- SBUF tiles: one per input transfer ----------------------------
    HQ0 = consts.tile([128, 640], b16)    # [AT kt0 | queries 0:512]
    HQ1 = consts.tile([128, 640], b16)    # [AT kt1 | queries 512:1024]
    G0 = consts.tile([128, 1024], b16)    # [seqn kt0-1 | AT kt2..7]
    AT2 = consts.tile([128, 1024], b16)   # AT kt8..15
    SNB = consts.tile([128, 768], b16)    # seqn kt2..7
    SNC = consts.tile([128, 1024], b16)   # seqn kt8..15
    warm_sb = consts.tile([128, QC], b16)

    def at_sl(kt):
        if kt == 0:
            return HQ0[:, 0:128]
        if kt == 1:
            return HQ1[:, 0:128]
        if kt < 8:
            return G0[:, 256 + (kt - 2) * 128:256 + (kt - 1) * 128]
        return AT2[:, (kt - 8) * 128:(kt - 7) * 128]

    def sn_sl(kt):
        if kt < 2:
            return G0[:, kt * 128:(kt + 1) * 128]
        if kt < 8:
            return SNB[:, (kt - 2) * 128:(kt - 1) * 128]
        return SNC[:, (kt - 8) * 128:(kt - 7) * 128]

    qrhs = [HQ0[:, 128:640], HQ1[:, 128:640]]

    # ---- input DMAs: one chain-critical transfer FIRST on each ring ----
    # (all rings share the 16 SDMA engines packet-round-robin, so any
    # second-slot transfer competes with global traffic; each transfer
    # also pays ~0.8us completion-semaphore latency after its last byte)
    nc.sync.dma_start(HQ0[:], ins["h0"])
    nc.scalar.dma_start(HQ1[:], ins["h1"])
    nc.gpsimd.dma_start(G0[:], ins["g0"])
    nc.sync.dma_start(AT2[:], ins["at2"])
    nc.scalar.dma_start(SNB[:], ins["snb"])
    nc.gpsimd.dma_start(SNC[:], ins["snc"])

    # warm-up matmuls: keep PE busy through the DMA head so the HAM
    # clock-gate releases (1.2 -> 2.4 GHz) before the real matmul stream
    nc.vector.memset(warm_sb[:], 1.0)
    for w in range(5):
        pw = psum.tile([128, QC], f32, tag="mm", bufs=3, name=f"pw{w}")
        nc.tensor.matmul(pw[:], warm_sb[:, 0:128], warm_sb[:],
                         start=True, stop=True)

    # ---- C accumulation banks ------------------------------------------
    pcs = [psum.tile([128, QC], f32, tag="ctx", bufs=2, name=f"pc{qc}")
           for qc in range(2)]

    def score_half(kt, qc, et_dst, name):
        # single [128,512] scores matmul + exp (first / last key tile)
        pp = psum.tile([128, QC], f32, tag="mm", bufs=3, name=name)
        nc.tensor.matmul(pp[:], at_sl(kt), qrhs[qc], start=True, stop=True)
        nc.scalar.activation(et_dst, pp[:], Exp, scale=float(SCALE))

    def score_tile(kt, et_dst):
        pp = psum.tile([128, 1024], f32, tag="mm", bufs=3, name=f"pp{kt}")
        for qc in range(2):
            nc.tensor.matmul(pp[:, qc * QC:(qc + 1) * QC], at_sl(kt),
                             qrhs[qc], start=True, stop=True,
                             skip_group_check=True)
        nc.scalar.activation(et_dst, pp[:], Exp, scale=float(SCALE))

    def c_mm(kt, et_sl, first=False, last=False):
        for qc in range(2):
            nc.tensor.matmul(pcs[qc][:], sn_sl(kt),
                             et_sl[:, qc * QC:(qc + 1) * QC],
                             start=first, stop=last)

    # et storage: kt0 / kt15 as standalone [128,1024]; kt1..14 as pairs
    et0 = et_pool.tile([128, 1024], b16, tag="et", name="et0")
    et15 = et_pool.tile([128, 1024], b16, tag="et", name="et15")
    prs = [et_pool.tile([128, 2048], b16, tag="et", name=f"etp{p}")
           for p in range(7)]
    esl = {0: et0, 15: et15}
    for kt in range(1, 15):
        p, half = (kt - 1) // 2, (kt - 1) % 2
        esl[kt] = prs[p][:, half * 1024:(half + 1) * 1024]

    accP = acc_pool.tile([128, 2048], b16, tag="acc", name="accP")
    accF = acc_pool.tile([128, 1024], b16, tag="acc", name="accF")

    # ---- main stream: scores/exp lead, C matmuls lag one kt ------------
    score_half(0, 0, et0[:, 0:QC], "pp0a")
    score_half(0, 1, et0[:, QC:1024], "pp0b")
    for kt in range(1, 15):
        score_tile(kt, esl[kt])
        c_mm(kt - 1, esl[kt - 1], first=(kt == 1))
        # DVE pair-accumulation, woven in as pairs complete (covers et0..12)
        if kt == 5:
            nc.vector.tensor_tensor(accP[:], prs[0][:], prs[1][:], add)
            nc.vector.tensor_tensor(accP[:, 0:1024], accP[:, 0:1024],
                                    et0[:], add)
        elif kt in (7, 9, 11, 13):
            nc.vector.tensor_tensor(accP[:], accP[:], prs[(kt - 3) // 2][:],
                                    add)
    score_half(15, 0, et15[:, 0:QC], "pp15a")
    c_mm(14, esl[14])
    nc.tensor.matmul(pcs[0][:], sn_sl(15), et15[:, 0:QC],
                     start=False, stop=True)
    score_half(15, 1, et15[:, QC:1024], "pp15b")
    nc.tensor.matmul(pcs[1][:], sn_sl(15), et15[:, QC:1024],
                     start=False, stop=True)

    # ---- sumexp partials: fold early; only et15 halves trail the chain -
    nc.vector.tensor_tensor(accF[:], accP[:, 0:1024], accP[:, 1024:2048], add)
    nc.vector.tensor_tensor(accF[:], accF[:], prs[6][:, 0:1024], add)   # et13
    nc.vector.tensor_tensor(accF[:], accF[:], prs[6][:, 1024:2048], add)  # et14
    nc.vector.tensor_tensor(accF[:, 0:QC], accF[:, 0:QC], et15[:, 0:QC], add)
    nc.vector.tensor_tensor(accF[:, QC:1024], accF[:, QC:1024],
                            et15[:, QC:1024], add)
    nc.gpsimd.dma_start(accf_d[:], accF[:])

    # ---- outputs: C halves on scalar/vector + their two rings ----------
    C0_sb = c_pool.tile([128, QC], b16, tag="c", name="C0")
    nc.scalar.copy(C0_sb[:], pcs[0][:])
    nc.scalar.dma_start(outT_d[:, 0:QC], C0_sb[:])

    C1_sb = c_pool.tile([128, QC], b16, tag="c", name="C1")
    nc.vector.tensor_copy(C1_sb[:], pcs[1][:])
    nc.sync.dma_start(outT_d[:, QC:2 * QC], C1_sb[:])


def _build_nc():
    from contextlib import ExitStack

    import concourse.mybir as mybir
    import concourse.tile as tile
    from concourse import bacc

    b16 = mybir.dt.bfloat16
    nc = bacc.Bacc("TRN2", target_bir_lowering=False, debug=False, num_devices=8)
    shapes = {
        "h0": [128, 640], "h1": [128, 640], "g0": [128, 1024],
        "at2": [128, 1024], "snb": [128, 768], "snc": [128, 1024],
    }
    ins = {k: nc.dram_tensor(k, sh, b16, kind="ExternalInput").ap()
           for k, sh in shapes.items()}
    outT_d = nc.dram_tensor("outT", [128, QPC], b16, kind="ExternalOutput").ap()
    accf_d = nc.dram_tensor("accf", [128, QPC], b16, kind="ExternalOutput").ap()

    with tile.TileContext(nc) as tc:
        with ExitStack() as ctx:
            _body(ctx, tc, ins, outT_d, accf_d)
    nc.compile()
    return nc


def get_nc():
    global _NC
    if _NC is None:
        _NC = _build_nc()
    return _NC


def make_in_maps(sequence, w_qkv):
    import ml_dtypes

    bf16 = ml_dtypes.bfloat16
    wq, wk = w_qkv[:O], w_qkv[O:2 * O]
    M = wk.T @ wq                                     # [128, 128]

    in_maps = []
    for b in range(B):
        seq = sequence[b]                             # [2048, 128] fp32
        AT = np.ascontiguousarray((seq @ M).T.astype(bf16))   # [128, 2048]
        seq16 = seq.astype(bf16)
        seqT = np.ascontiguousarray(seq16.T)          # [128, 2048]
        # seqn tiled: partition p holds [t, i] for key t*128+p
        seqn = np.ascontiguousarray(
            seq16.reshape(NKT, 128, 128).transpose(1, 0, 2).reshape(128, S))
        at2 = np.ascontiguousarray(AT[:, 1024:2048])
        snb = np.ascontiguousarray(seqn[:, 256:1024])
        snc = np.ascontiguousarray(seqn[:, 1024:2048])
        for h in range(2):
            q = seqT[:, h * QPC:(h + 1) * QPC]
            in_maps.append({
                "h0": np.ascontiguousarray(
                    np.concatenate([AT[:, 0:128], q[:, 0:QC]], axis=1)),
                "h1": np.ascontiguousarray(
                    np.concatenate([AT[:, 128:256], q[:, QC:QPC]], axis=1)),
                "g0": np.ascontiguousarray(
                    np.concatenate([seqn[:, 0:256], AT[:, 256:1024]], axis=1)),
                "at2": at2, "snb": snb, "snc": snc,
            })
    return in_maps


def kernel(sequence, w_qkv, w_out, b_out):
    global LAST_RESULTS
    from concourse.bass_utils import run_bass_kernel_spmd

    sequence = np.asarray(sequence, dtype=np.float32)
    w_qkv = np.asarray(w_qkv, dtype=np.float32)
    w_out = np.asarray(w_out, dtype=np.float32)
    b_out = np.asarray(b_out, dtype=np.float32)

    nc = get_nc()
    in_maps = make_in_maps(sequence, w_qkv)
    kw = {}
    if PROFILE:
        kw = dict(trace=True, trace_cores=[0])
    res = run_bass_kernel_spmd(nc, in_maps, list(range(8)), **kw)
    LAST_RESULTS = res

    wv = w_qkv[2 * O:]
    W2T = (wv.T @ w_out.T).astype(np.float32)              # [128, 128]
    out = np.empty((B, S, DIN), np.float32)
    for c in range(8):
        b, h = c // 2, c % 2
        C = res.results[c]["outT"].astype(np.float32)      # [128,1024] seqT E^T
        se = res.results[c]["accf"].astype(np.float32).sum(axis=0)  # [1024]
        outT = W2T.T @ C                                   # [128, 1024]
        out[b, h * QPC:(h + 1) * QPC, :] = outT.T / se[:, None] + b_out[None, :]
    return out
